# revision 7
# baseline (speedup 1.0000x reference)
"""GNN message-passing (MGN mailbox sum + Linear + indeg blend) on 8 Trainium2 cores.

Reference semantics (for full inputs h[40000,128], W[128,128], b[128],
src/dst[640000]):
    agg     = segment_sum(h[src], dst, 40000)
    updated = agg @ W.T + b
    out     = where(indeg > 0, updated, h)

Sharding (per the problem's sharding hint): edges and their *gathered
features* are sharded across the 8 cores by destination-node range; the
Linear weight is replicated. Each core owns 5120 destination nodes (40
windows of 128). The host buckets edges by destination window (a sort by
dst) and ships each core the pre-gathered edge features h[src] (bf16) in a
fixed [window, tile, slot] layout, plus per-slot one-hot column indices.

Device compute per window w (40 per core):
    O_w   = onehot(dst_local)          # GpSimd local_scatter (2 halves)
    aggT  = sum_t stage_t.T @ O_t      # PE, PSUM accumulate   [128f, 128n]
    updT  = W @ aggT                   # PE (replicated W)     [128o, 128n]
    updT += b                          # ACT Identity+bias
    outT  = where(maskT, updT, hT)     # DVE copy_predicated, in-place in the
                                       #   resident hT buffer
Everything stays feature-major (no on-chip transposes); the host
transposes each core's [128, 5120] result back at the end.

Slots beyond a window's edge count get one-hot column -1 (not written ->
zero one-hot row). If a window exceeds the T*128 slot capacity (6-sigma
event), the affected destination nodes are recomputed exactly on the host
and patched into the output.
"""

import sys

sys.path.insert(0, "/opt/trn_rl_repo")

import numpy as np
import ml_dtypes

import concourse.bacc as bacc
import concourse.mybir as mybir
import concourse.tile as tile
from concourse.bass_utils import run_bass_kernel_spmd

BF16 = ml_dtypes.bfloat16

# problem geometry (hardcoded per spec)
N_NODES = 40000
N_EDGES = 640000
HID = 128
P = 128

N_CORES = 8
PAD_NODES = 40960           # 8 cores x 40 windows x 128 nodes
NPC = PAD_NODES // N_CORES  # 5120 nodes per core
WPC = NPC // P              # 40 windows per core
T = 18                      # edge tiles per window (capacity T*128 = 2304, mean 2048)
TH = T // 2                 # tiles per one-hot half
NIX = TH + 1                # local_scatter num_idxs per half (padded to even)
GRP = 4                     # windows fused per Linear/bias/blend batch (512 cols)

_NC_CACHE = {}


def _build_nc():
    """Build the (shared, SPMD) bass program. Same program runs on all 8 cores."""
    key = "v6"
    if key in _NC_CACHE:
        return _NC_CACHE[key]
    f32 = mybir.dt.float32
    bf16 = mybir.dt.bfloat16
    i16 = mybir.dt.int16
    nc = bacc.Bacc(None, target_bir_lowering=False)

    stage = nc.declare_dram_parameter("stage", [P, WPC * T * P], bf16, isOutput=False)
    colix = nc.declare_dram_parameter("colix", [P, WPC * 2 * NIX], i16, isOutput=False)
    dl = nc.declare_dram_parameter("dl", [P, WPC * T], bf16, isOutput=False)
    iota = nc.declare_dram_parameter("iota", [P, P], bf16, isOutput=False)
    wt = nc.declare_dram_parameter("wt", [P, P], bf16, isOutput=False)
    b2 = nc.declare_dram_parameter("b2", [P, 1], f32, isOutput=False)
    hT = nc.declare_dram_parameter("hT", [P, NPC], f32, isOutput=False)
    maskT = nc.declare_dram_parameter("maskT", [P, NPC], mybir.dt.uint8, isOutput=False)
    outT = nc.declare_dram_parameter("outT", [P, NPC], f32, isOutput=True)

    with tile.TileContext(nc) as tc:
        with (
            tc.tile_pool(name="const", bufs=1) as constp,
            tc.tile_pool(name="big", bufs=1) as bigp,
            tc.tile_pool(name="stagep", bufs=5) as stagep,
            tc.tile_pool(name="onehotp", bufs=8) as onehotp,
            tc.tile_pool(name="smallp", bufs=6) as smallp,
            tc.tile_pool(name="psA", bufs=4, space="PSUM") as psA,
            tc.tile_pool(name="psB", bufs=2, space="PSUM") as psB,
        ):
            wt_t = constp.tile([P, P], bf16)
            nc.sync.dma_start(out=wt_t[:], in_=wt[:])
            b2_t = constp.tile([P, 1], f32)
            nc.sync.dma_start(out=b2_t[:], in_=b2[:])
            ones_t = constp.tile([P, NIX], bf16)
            nc.vector.memset(ones_t[:], 1.0)
            cix_t = constp.tile([P, WPC * 2 * NIX], i16)
            nc.sync.dma_start(out=cix_t[:], in_=colix[:])
            iota_t = constp.tile([P, P], bf16)
            nc.sync.dma_start(out=iota_t[:], in_=iota[:])
            dl_t = constp.tile([P, WPC * T], bf16)
            nc.sync.dma_start(out=dl_t[:], in_=dl[:])

            hT_buf = bigp.tile([P, NPC], f32)
            nc.sync.dma_start(out=hT_buf[:], in_=hT[:])
            mk_buf = bigp.tile([P, NPC], mybir.dt.uint8)
            nc.sync.dma_start(out=mk_buf[:], in_=maskT[:])

            for w in range(WPC):
                st = stagep.tile([P, T * P], bf16, tag="stage")
                nc.sync.dma_start(out=st[:], in_=stage[:, w * T * P : (w + 1) * T * P])

                oh = []
                for half in range(2):
                    o = onehotp.tile([P, TH * P], bf16, tag=f"oh{half}")
                    if w % 5 < 2:
                        # DVE path: is_equal(dst_local, iota)
                        nc.vector.tensor_tensor(
                            out=o[:].rearrange("p (t f) -> p t f", f=P),
                            in0=dl_t[
                                :, w * T + half * TH : w * T + (half + 1) * TH, None
                            ].to_broadcast([P, TH, P]),
                            in1=iota_t[:, None, :].to_broadcast([P, TH, P]),
                            op=mybir.AluOpType.is_equal,
                        )
                    else:
                        # GpSimd path: scatter ones at precomputed columns
                        nc.gpsimd.local_scatter(
                            out_ap=o[:],
                            data_ap=ones_t[:],
                            idxs_ap=cix_t[
                                :, (w * 2 + half) * NIX : (w * 2 + half + 1) * NIX
                            ],
                            channels=P,
                            num_elems=TH * P,
                            num_idxs=NIX,
                        )
                    oh.append(o)

                paggT = psA.tile([P, P], f32, tag="paggT")
                for t in range(T):
                    o = oh[t // TH]
                    tl = t % TH
                    nc.tensor.matmul(
                        out=paggT[:],
                        lhsT=st[:, t * P : (t + 1) * P],
                        rhs=o[:, tl * P : (tl + 1) * P],
                        start=(t == 0),
                        stop=(t == T - 1),
                    )
                wi = w % GRP
                if wi == 0:
                    aggT4 = smallp.tile([P, GRP * P], bf16, tag="aggT")
                nc.scalar.copy(
                    out=aggT4[:, wi * P : (wi + 1) * P], in_=paggT[:]
                )

                if wi == GRP - 1:
                    g0 = (w - GRP + 1) * P
                    pupdT = psB.tile([P, GRP * P], f32, tag="pupdT")
                    nc.tensor.matmul(
                        out=pupdT[:], lhsT=wt_t[:], rhs=aggT4[:], start=True, stop=True
                    )
                    updT_s = smallp.tile([P, GRP * P], f32, tag="updT")
                    nc.scalar.activation(
                        out=updT_s[:],
                        in_=pupdT[:],
                        func=mybir.ActivationFunctionType.Identity,
                        bias=b2_t[:, :1],
                    )
                    nc.vector.copy_predicated(
                        hT_buf[:, g0 : g0 + GRP * P],
                        mk_buf[:, g0 : g0 + GRP * P],
                        updT_s[:],
                    )

            nc.sync.dma_start(out=outT[:], in_=hT_buf[:])

    nc.finalize()
    _NC_CACHE[key] = nc
    return nc


def kernel(h, W, b, src, dst):
    h = np.ascontiguousarray(np.asarray(h, dtype=np.float32))
    W = np.ascontiguousarray(np.asarray(W, dtype=np.float32))
    b = np.ascontiguousarray(np.asarray(b, dtype=np.float32))
    src = np.asarray(src).astype(np.int64)
    dst = np.asarray(dst).astype(np.int64)
    n, hid = h.shape
    assert (n, hid) == (N_NODES, HID)

    h_pad = np.zeros((PAD_NODES + 1, HID), np.float32)  # +1: row PAD_NODES = zero row
    h_pad[:N_NODES] = h
    h_pad_bf = h_pad.astype(BF16)

    # ---- host-side sharding: bucket edges by dst window, fixed-capacity slots
    order = np.argsort(dst, kind="stable")
    dst_s = dst[order]
    src_s = src[order]
    win_bounds = np.searchsorted(dst_s, np.arange(0, PAD_NODES + P, P))
    cap = T * P

    n_win = PAD_NODES // P  # 320
    spill_nodes = []
    slot_src = np.full((n_win, cap), PAD_NODES, np.int64)  # default: zero row
    slot_dl = np.full((n_win, cap), -1, np.int64)          # -1: empty slot
    for wgl in range(n_win):
        lo, hi = win_bounds[wgl], win_bounds[wgl + 1]
        cnt = hi - lo
        take = min(cnt, cap)
        slot_src[wgl, :take] = src_s[lo : lo + take]
        slot_dl[wgl, :take] = dst_s[lo : lo + take] - wgl * P
        if cnt > cap:
            spill_nodes.append(np.unique(dst_s[lo + cap : hi]))

    indeg = np.bincount(dst, minlength=PAD_NODES)

    # one-hot column indices per slot: col = (tile % TH) * 128 + dst_local
    # shipped layout: [P, WPC * 2 * NIX] int16; per (window, half): NIX entries
    # per partition (tile-within-half 0..TH-1, then one padding -1)
    sl = slot_dl.reshape(n_win, T, P)  # [win, tile, part]
    colix_all = np.full((n_win, 2, NIX, P), -1, np.int64)
    for half in range(2):
        tl = sl[:, half * TH : (half + 1) * TH, :]  # [win, TH, part]
        colix_all[:, half, :TH, :] = np.where(
            tl >= 0, (np.arange(TH)[None, :, None]) * P + tl, -1
        )

    WT = np.ascontiguousarray(W.T).astype(BF16)
    b2 = np.ascontiguousarray(b[:, None])
    iota_np = np.tile(np.arange(P, dtype=np.float32), (P, 1)).astype(BF16)

    in_maps = []
    for c in range(N_CORES):
        wsl = slice(c * WPC, (c + 1) * WPC)
        rows = h_pad_bf[slot_src[wsl]]  # [WPC, cap, HID]
        rows = rows.reshape(WPC, T, P, HID)
        stage_np = np.ascontiguousarray(
            rows.transpose(2, 0, 1, 3).reshape(P, WPC * T * P)
        )
        colix_np = np.ascontiguousarray(
            colix_all[wsl].transpose(3, 0, 1, 2).reshape(P, WPC * 2 * NIX)
        ).astype(np.int16)
        dl_np = np.ascontiguousarray(
            np.where(slot_dl[wsl] >= 0, slot_dl[wsl], 255)
            .reshape(WPC, T, P)
            .transpose(2, 0, 1)
            .reshape(P, WPC * T)
            .astype(np.float32)
        ).astype(BF16)
        hT_np = np.ascontiguousarray(h_pad[c * NPC : (c + 1) * NPC, :].T)
        maskT_np = np.ascontiguousarray(
            np.broadcast_to(
                (indeg[c * NPC : (c + 1) * NPC] > 0).astype(np.uint8)[None, :],
                (P, NPC),
            )
        )
        in_maps.append(
            {
                "stage": stage_np,
                "colix": colix_np,
                "dl": dl_np,
                "iota": iota_np,
                "wt": WT,
                "b2": b2,
                "hT": hT_np,
                "maskT": maskT_np,
            }
        )

    nc = _build_nc()
    res = run_bass_kernel_spmd(nc, in_maps, core_ids=list(range(N_CORES)))

    out = np.concatenate([res.results[c]["outT"].T for c in range(N_CORES)], axis=0)
    out = np.ascontiguousarray(out[:N_NODES])

    # ---- host patch for (statistically negligible) window-capacity spill
    if spill_nodes:
        nodes = np.unique(np.concatenate(spill_nodes))
        nodes = nodes[nodes < N_NODES]
        if nodes.size:
            sel = np.isin(dst, nodes)
            agg = np.zeros((nodes.size, HID), np.float32)
            remap = {int(v): i for i, v in enumerate(nodes)}
            np.add.at(agg, [remap[int(d)] for d in dst[sel]], h[src[sel]])
            out[nodes] = agg @ W.T + b

    return out


# revision 8
# speedup vs baseline: 1.0990x; 1.0990x over previous
"""GNN message-passing (MGN mailbox sum + Linear + indeg blend) on 8 Trainium2 cores.

Reference semantics (for full inputs h[40000,128], W[128,128], b[128],
src/dst[640000]):
    agg     = segment_sum(h[src], dst, 40000)
    updated = agg @ W.T + b
    out     = where(indeg > 0, updated, h)

Sharding (per the problem's sharding hint): edges and their *gathered
features* are sharded across the 8 cores by destination-node range; the
Linear weight is replicated. Each core owns 5120 destination nodes (40
windows of 128). The host buckets edges by destination window (a sort by
dst) and ships each core the pre-gathered edge features h[src] (bf16) in a
fixed [window, tile, slot] layout, plus per-slot one-hot column indices.

Device compute per window w (40 per core):
    O_w   = onehot(dst_local)          # GpSimd local_scatter (2 halves)
    aggT  = sum_t stage_t.T @ O_t      # PE, PSUM accumulate   [128f, 128n]
    updT  = W @ aggT                   # PE (replicated W)     [128o, 128n]
    updT += b                          # ACT Identity+bias
    outT  = where(maskT, updT, hT)     # DVE copy_predicated, in-place in the
                                       #   resident hT buffer
Everything stays feature-major (no on-chip transposes); the host
transposes each core's [128, 5120] result back at the end.

Slots beyond a window's edge count get one-hot column -1 (not written ->
zero one-hot row). If a window exceeds the T*128 slot capacity (6-sigma
event), the affected destination nodes are recomputed exactly on the host
and patched into the output.
"""

import sys

sys.path.insert(0, "/opt/trn_rl_repo")

import numpy as np
import ml_dtypes

import concourse.bacc as bacc
import concourse.mybir as mybir
import concourse.tile as tile
from concourse.bass_utils import run_bass_kernel_spmd

BF16 = ml_dtypes.bfloat16

# problem geometry (hardcoded per spec)
N_NODES = 40000
N_EDGES = 640000
HID = 128
P = 128

N_CORES = 8
PAD_NODES = 40960           # 8 cores x 40 windows x 128 nodes
NPC = PAD_NODES // N_CORES  # 5120 nodes per core
WPC = NPC // P              # 40 windows per core
T = 18                      # edge tiles per window (capacity T*128 = 2304, mean 2048)
TH = T // 2                 # tiles per one-hot half
NIX = TH + 1                # local_scatter num_idxs per half (padded to even)
GRP = 2                     # windows fused per Linear/bias/blend batch (512 cols)

_NC_CACHE = {}


def _build_nc():
    """Build the (shared, SPMD) bass program. Same program runs on all 8 cores."""
    key = "v6b"
    if key in _NC_CACHE:
        return _NC_CACHE[key]
    f32 = mybir.dt.float32
    bf16 = mybir.dt.bfloat16
    i16 = mybir.dt.int16
    nc = bacc.Bacc(None, target_bir_lowering=False)

    stage = nc.declare_dram_parameter("stage", [P, WPC * T * P], bf16, isOutput=False)
    colix = nc.declare_dram_parameter("colix", [P, WPC * 2 * NIX], i16, isOutput=False)
    dl = nc.declare_dram_parameter("dl", [P, WPC * T], bf16, isOutput=False)
    iota = nc.declare_dram_parameter("iota", [P, P], bf16, isOutput=False)
    wt = nc.declare_dram_parameter("wt", [P, P], bf16, isOutput=False)
    b2 = nc.declare_dram_parameter("b2", [P, 1], f32, isOutput=False)
    hT = nc.declare_dram_parameter("hT", [P, NPC], f32, isOutput=False)
    maskT = nc.declare_dram_parameter("maskT", [P, NPC], mybir.dt.uint8, isOutput=False)
    outT = nc.declare_dram_parameter("outT", [P, NPC], f32, isOutput=True)

    with tile.TileContext(nc) as tc:
        with (
            tc.tile_pool(name="const", bufs=1) as constp,
            tc.tile_pool(name="big", bufs=1) as bigp,
            tc.tile_pool(name="stagep", bufs=5) as stagep,
            tc.tile_pool(name="onehotp", bufs=8) as onehotp,
            tc.tile_pool(name="smallp", bufs=6) as smallp,
            tc.tile_pool(name="psA", bufs=4, space="PSUM") as psA,
            tc.tile_pool(name="psB", bufs=2, space="PSUM") as psB,
        ):
            wt_t = constp.tile([P, P], bf16)
            nc.sync.dma_start(out=wt_t[:], in_=wt[:])
            b2_t = constp.tile([P, 1], f32)
            nc.sync.dma_start(out=b2_t[:], in_=b2[:])
            ones_t = constp.tile([P, NIX], bf16)
            nc.vector.memset(ones_t[:], 1.0)
            cix_t = constp.tile([P, WPC * 2 * NIX], i16)
            nc.sync.dma_start(out=cix_t[:], in_=colix[:])
            iota_t = constp.tile([P, P], bf16)
            nc.sync.dma_start(out=iota_t[:], in_=iota[:])
            dl_t = constp.tile([P, WPC * T], bf16)
            nc.sync.dma_start(out=dl_t[:], in_=dl[:])

            hT_buf = bigp.tile([P, NPC], f32)
            nc.sync.dma_start(out=hT_buf[:], in_=hT[:])
            mk_buf = bigp.tile([P, NPC], mybir.dt.uint8)
            nc.sync.dma_start(out=mk_buf[:], in_=maskT[:])

            for w in range(WPC):
                st = stagep.tile([P, T * P], bf16, tag="stage")
                nc.sync.dma_start(out=st[:], in_=stage[:, w * T * P : (w + 1) * T * P])

                oh = []
                for half in range(2):
                    o = onehotp.tile([P, TH * P], bf16, tag=f"oh{half}")
                    if w % 5 < 2:
                        # DVE path: is_equal(dst_local, iota)
                        nc.vector.tensor_tensor(
                            out=o[:].rearrange("p (t f) -> p t f", f=P),
                            in0=dl_t[
                                :, w * T + half * TH : w * T + (half + 1) * TH, None
                            ].to_broadcast([P, TH, P]),
                            in1=iota_t[:, None, :].to_broadcast([P, TH, P]),
                            op=mybir.AluOpType.is_equal,
                        )
                    else:
                        # GpSimd path: scatter ones at precomputed columns
                        nc.gpsimd.local_scatter(
                            out_ap=o[:],
                            data_ap=ones_t[:],
                            idxs_ap=cix_t[
                                :, (w * 2 + half) * NIX : (w * 2 + half + 1) * NIX
                            ],
                            channels=P,
                            num_elems=TH * P,
                            num_idxs=NIX,
                        )
                    oh.append(o)

                paggT = psA.tile([P, P], f32, tag="paggT")
                for t in range(T):
                    o = oh[t // TH]
                    tl = t % TH
                    nc.tensor.matmul(
                        out=paggT[:],
                        lhsT=st[:, t * P : (t + 1) * P],
                        rhs=o[:, tl * P : (tl + 1) * P],
                        start=(t == 0),
                        stop=(t == T - 1),
                    )
                wi = w % GRP
                if wi == 0:
                    aggT4 = smallp.tile([P, GRP * P], bf16, tag="aggT")
                nc.scalar.copy(
                    out=aggT4[:, wi * P : (wi + 1) * P], in_=paggT[:]
                )

                if wi == GRP - 1:
                    g0 = (w - GRP + 1) * P
                    pupdT = psB.tile([P, GRP * P], f32, tag="pupdT")
                    nc.tensor.matmul(
                        out=pupdT[:], lhsT=wt_t[:], rhs=aggT4[:], start=True, stop=True
                    )
                    updT_s = smallp.tile([P, GRP * P], f32, tag="updT")
                    nc.scalar.activation(
                        out=updT_s[:],
                        in_=pupdT[:],
                        func=mybir.ActivationFunctionType.Identity,
                        bias=b2_t[:, :1],
                    )
                    nc.vector.copy_predicated(
                        hT_buf[:, g0 : g0 + GRP * P],
                        mk_buf[:, g0 : g0 + GRP * P],
                        updT_s[:],
                    )

            nc.sync.dma_start(out=outT[:], in_=hT_buf[:])

    nc.finalize()
    _NC_CACHE[key] = nc
    return nc


def kernel(h, W, b, src, dst):
    h = np.ascontiguousarray(np.asarray(h, dtype=np.float32))
    W = np.ascontiguousarray(np.asarray(W, dtype=np.float32))
    b = np.ascontiguousarray(np.asarray(b, dtype=np.float32))
    src = np.asarray(src).astype(np.int64)
    dst = np.asarray(dst).astype(np.int64)
    n, hid = h.shape
    assert (n, hid) == (N_NODES, HID)

    h_pad = np.zeros((PAD_NODES + 1, HID), np.float32)  # +1: row PAD_NODES = zero row
    h_pad[:N_NODES] = h
    h_pad_bf = h_pad.astype(BF16)

    # ---- host-side sharding: bucket edges by dst window, fixed-capacity slots
    order = np.argsort(dst, kind="stable")
    dst_s = dst[order]
    src_s = src[order]
    win_bounds = np.searchsorted(dst_s, np.arange(0, PAD_NODES + P, P))
    cap = T * P

    n_win = PAD_NODES // P  # 320
    spill_nodes = []
    slot_src = np.full((n_win, cap), PAD_NODES, np.int64)  # default: zero row
    slot_dl = np.full((n_win, cap), -1, np.int64)          # -1: empty slot
    for wgl in range(n_win):
        lo, hi = win_bounds[wgl], win_bounds[wgl + 1]
        cnt = hi - lo
        take = min(cnt, cap)
        slot_src[wgl, :take] = src_s[lo : lo + take]
        slot_dl[wgl, :take] = dst_s[lo : lo + take] - wgl * P
        if cnt > cap:
            spill_nodes.append(np.unique(dst_s[lo + cap : hi]))

    indeg = np.bincount(dst, minlength=PAD_NODES)

    # one-hot column indices per slot: col = (tile % TH) * 128 + dst_local
    # shipped layout: [P, WPC * 2 * NIX] int16; per (window, half): NIX entries
    # per partition (tile-within-half 0..TH-1, then one padding -1)
    sl = slot_dl.reshape(n_win, T, P)  # [win, tile, part]
    colix_all = np.full((n_win, 2, NIX, P), -1, np.int64)
    for half in range(2):
        tl = sl[:, half * TH : (half + 1) * TH, :]  # [win, TH, part]
        colix_all[:, half, :TH, :] = np.where(
            tl >= 0, (np.arange(TH)[None, :, None]) * P + tl, -1
        )

    WT = np.ascontiguousarray(W.T).astype(BF16)
    b2 = np.ascontiguousarray(b[:, None])
    iota_np = np.tile(np.arange(P, dtype=np.float32), (P, 1)).astype(BF16)

    in_maps = []
    for c in range(N_CORES):
        wsl = slice(c * WPC, (c + 1) * WPC)
        rows = h_pad_bf[slot_src[wsl]]  # [WPC, cap, HID]
        rows = rows.reshape(WPC, T, P, HID)
        stage_np = np.ascontiguousarray(
            rows.transpose(2, 0, 1, 3).reshape(P, WPC * T * P)
        )
        colix_np = np.ascontiguousarray(
            colix_all[wsl].transpose(3, 0, 1, 2).reshape(P, WPC * 2 * NIX)
        ).astype(np.int16)
        dl_np = np.ascontiguousarray(
            np.where(slot_dl[wsl] >= 0, slot_dl[wsl], 255)
            .reshape(WPC, T, P)
            .transpose(2, 0, 1)
            .reshape(P, WPC * T)
            .astype(np.float32)
        ).astype(BF16)
        hT_np = np.ascontiguousarray(h_pad[c * NPC : (c + 1) * NPC, :].T)
        maskT_np = np.ascontiguousarray(
            np.broadcast_to(
                (indeg[c * NPC : (c + 1) * NPC] > 0).astype(np.uint8)[None, :],
                (P, NPC),
            )
        )
        in_maps.append(
            {
                "stage": stage_np,
                "colix": colix_np,
                "dl": dl_np,
                "iota": iota_np,
                "wt": WT,
                "b2": b2,
                "hT": hT_np,
                "maskT": maskT_np,
            }
        )

    nc = _build_nc()
    res = run_bass_kernel_spmd(nc, in_maps, core_ids=list(range(N_CORES)))

    out = np.concatenate([res.results[c]["outT"].T for c in range(N_CORES)], axis=0)
    out = np.ascontiguousarray(out[:N_NODES])

    # ---- host patch for (statistically negligible) window-capacity spill
    if spill_nodes:
        nodes = np.unique(np.concatenate(spill_nodes))
        nodes = nodes[nodes < N_NODES]
        if nodes.size:
            sel = np.isin(dst, nodes)
            agg = np.zeros((nodes.size, HID), np.float32)
            remap = {int(v): i for i, v in enumerate(nodes)}
            np.add.at(agg, [remap[int(d)] for d in dst[sel]], h[src[sel]])
            out[nodes] = agg @ W.T + b

    return out


# revision 9
# speedup vs baseline: 1.1005x; 1.0013x over previous
"""GNN message-passing (MGN mailbox sum + Linear + indeg blend) on 8 Trainium2 cores.

Reference semantics (for full inputs h[40000,128], W[128,128], b[128],
src/dst[640000]):
    agg     = segment_sum(h[src], dst, 40000)
    updated = agg @ W.T + b
    out     = where(indeg > 0, updated, h)

Sharding (per the problem's sharding hint): edges and their *gathered
features* are sharded across the 8 cores by destination-node range; the
Linear weight is replicated. Each core owns 5120 destination nodes (40
windows of 128). The host buckets edges by destination window (a sort by
dst) and ships each core the pre-gathered edge features h[src] (bf16) in a
fixed [window, tile, slot] layout, plus per-slot one-hot column indices.

Device compute per window w (40 per core):
    O_w   = onehot(dst_local)          # GpSimd local_scatter (2 halves)
    aggT  = sum_t stage_t.T @ O_t      # PE, PSUM accumulate   [128f, 128n]
    updT  = W @ aggT                   # PE (replicated W)     [128o, 128n]
    updT += b                          # ACT Identity+bias
    outT  = where(maskT, updT, hT)     # DVE copy_predicated, in-place in the
                                       #   resident hT buffer
Everything stays feature-major (no on-chip transposes); the host
transposes each core's [128, 5120] result back at the end.

Slots beyond a window's edge count get one-hot column -1 (not written ->
zero one-hot row). If a window exceeds the T*128 slot capacity (6-sigma
event), the affected destination nodes are recomputed exactly on the host
and patched into the output.
"""

import sys

sys.path.insert(0, "/opt/trn_rl_repo")

import numpy as np
import ml_dtypes

import concourse.bacc as bacc
import concourse.mybir as mybir
import concourse.tile as tile
from concourse.bass_utils import run_bass_kernel_spmd

BF16 = ml_dtypes.bfloat16

# problem geometry (hardcoded per spec)
N_NODES = 40000
N_EDGES = 640000
HID = 128
P = 128

N_CORES = 8
PAD_NODES = 40960           # 8 cores x 40 windows x 128 nodes
NPC = PAD_NODES // N_CORES  # 5120 nodes per core
WPC = NPC // P              # 40 windows per core
T = 18                      # edge tiles per window (capacity T*128 = 2304, mean 2048)
TH = T // 2                 # tiles per one-hot half
NIX = TH + 1                # local_scatter num_idxs per half (padded to even)
GRP = 2                     # windows fused per Linear/bias/blend batch (512 cols)

_NC_CACHE = {}


def _build_nc():
    """Build the (shared, SPMD) bass program. Same program runs on all 8 cores."""
    key = "v7"
    if key in _NC_CACHE:
        return _NC_CACHE[key]
    f32 = mybir.dt.float32
    bf16 = mybir.dt.bfloat16
    i16 = mybir.dt.int16
    nc = bacc.Bacc(None, target_bir_lowering=False)

    stage = nc.declare_dram_parameter("stage", [P, WPC * T * P], bf16, isOutput=False)
    colix = nc.declare_dram_parameter("colix", [P, WPC * 2 * NIX], i16, isOutput=False)
    dl = nc.declare_dram_parameter("dl", [P, WPC * T], bf16, isOutput=False)
    iota = nc.declare_dram_parameter("iota", [P, P], bf16, isOutput=False)
    wt = nc.declare_dram_parameter("wt", [P, P], bf16, isOutput=False)
    b2 = nc.declare_dram_parameter("b2", [P, 1], f32, isOutput=False)
    hT = nc.declare_dram_parameter("hT", [P, NPC], f32, isOutput=False)
    maskT = nc.declare_dram_parameter("maskT", [P, NPC], mybir.dt.uint8, isOutput=False)
    outT = nc.declare_dram_parameter("outT", [P, NPC], f32, isOutput=True)

    with tile.TileContext(nc) as tc:
        with (
            tc.tile_pool(name="const", bufs=1) as constp,
            tc.tile_pool(name="big", bufs=1) as bigp,
            tc.tile_pool(name="stagep", bufs=5) as stagep,
            tc.tile_pool(name="onehotp", bufs=8) as onehotp,
            tc.tile_pool(name="smallp", bufs=6) as smallp,
            tc.tile_pool(name="psA", bufs=4, space="PSUM") as psA,
            tc.tile_pool(name="psB", bufs=2, space="PSUM") as psB,
        ):
            wt_t = constp.tile([P, P], bf16)
            nc.sync.dma_start(out=wt_t[:], in_=wt[:])
            b2_t = constp.tile([P, 1], f32)
            nc.sync.dma_start(out=b2_t[:], in_=b2[:])
            ones_t = constp.tile([P, NIX], bf16)
            nc.vector.memset(ones_t[:], 1.0)
            cix_t = constp.tile([P, WPC * 2 * NIX], i16)
            nc.sync.dma_start(out=cix_t[:], in_=colix[:])
            iota_t = constp.tile([P, P], bf16)
            nc.sync.dma_start(out=iota_t[:], in_=iota[:])
            dl_t = constp.tile([P, WPC * T], bf16)
            nc.sync.dma_start(out=dl_t[:], in_=dl[:])

            hT_buf = bigp.tile([P, NPC], f32)
            nc.sync.dma_start(out=hT_buf[:], in_=hT[:])
            mk_buf = bigp.tile([P, NPC], mybir.dt.uint8)
            nc.sync.dma_start(out=mk_buf[:], in_=maskT[:])

            for w in range(WPC):
                st = stagep.tile([P, T * P], bf16, tag="stage")
                nc.sync.dma_start(out=st[:], in_=stage[:, w * T * P : (w + 1) * T * P])

                oh = []
                for half in range(2):
                    o = onehotp.tile([P, TH * P], bf16, tag=f"oh{half}")
                    if ((w * 2 + half) * 17) % 40 < 17:
                        # DVE path: is_equal(dst_local, iota)
                        nc.vector.tensor_tensor(
                            out=o[:].rearrange("p (t f) -> p t f", f=P),
                            in0=dl_t[
                                :, w * T + half * TH : w * T + (half + 1) * TH, None
                            ].to_broadcast([P, TH, P]),
                            in1=iota_t[:, None, :].to_broadcast([P, TH, P]),
                            op=mybir.AluOpType.is_equal,
                        )
                    else:
                        # GpSimd path: scatter ones at precomputed columns
                        nc.gpsimd.local_scatter(
                            out_ap=o[:],
                            data_ap=ones_t[:],
                            idxs_ap=cix_t[
                                :, (w * 2 + half) * NIX : (w * 2 + half + 1) * NIX
                            ],
                            channels=P,
                            num_elems=TH * P,
                            num_idxs=NIX,
                        )
                    oh.append(o)

                paggT = psA.tile([P, P], f32, tag="paggT")
                for t in range(T):
                    o = oh[t // TH]
                    tl = t % TH
                    nc.tensor.matmul(
                        out=paggT[:],
                        lhsT=st[:, t * P : (t + 1) * P],
                        rhs=o[:, tl * P : (tl + 1) * P],
                        start=(t == 0),
                        stop=(t == T - 1),
                    )
                wi = w % GRP
                if wi == 0:
                    aggT4 = smallp.tile([P, GRP * P], bf16, tag="aggT")
                nc.scalar.copy(
                    out=aggT4[:, wi * P : (wi + 1) * P], in_=paggT[:]
                )

                if wi == GRP - 1:
                    g0 = (w - GRP + 1) * P
                    pupdT = psB.tile([P, GRP * P], f32, tag="pupdT")
                    nc.tensor.matmul(
                        out=pupdT[:], lhsT=wt_t[:], rhs=aggT4[:], start=True, stop=True
                    )
                    updT_s = smallp.tile([P, GRP * P], f32, tag="updT")
                    nc.scalar.activation(
                        out=updT_s[:],
                        in_=pupdT[:],
                        func=mybir.ActivationFunctionType.Identity,
                        bias=b2_t[:, :1],
                    )
                    nc.vector.copy_predicated(
                        hT_buf[:, g0 : g0 + GRP * P],
                        mk_buf[:, g0 : g0 + GRP * P],
                        updT_s[:],
                    )

            nc.sync.dma_start(out=outT[:], in_=hT_buf[:])

    nc.finalize()
    _NC_CACHE[key] = nc
    return nc


def kernel(h, W, b, src, dst):
    h = np.ascontiguousarray(np.asarray(h, dtype=np.float32))
    W = np.ascontiguousarray(np.asarray(W, dtype=np.float32))
    b = np.ascontiguousarray(np.asarray(b, dtype=np.float32))
    src = np.asarray(src).astype(np.int64)
    dst = np.asarray(dst).astype(np.int64)
    n, hid = h.shape
    assert (n, hid) == (N_NODES, HID)

    h_pad = np.zeros((PAD_NODES + 1, HID), np.float32)  # +1: row PAD_NODES = zero row
    h_pad[:N_NODES] = h
    h_pad_bf = h_pad.astype(BF16)

    # ---- host-side sharding: bucket edges by dst window, fixed-capacity slots
    order = np.argsort(dst, kind="stable")
    dst_s = dst[order]
    src_s = src[order]
    win_bounds = np.searchsorted(dst_s, np.arange(0, PAD_NODES + P, P))
    cap = T * P

    n_win = PAD_NODES // P  # 320
    spill_nodes = []
    slot_src = np.full((n_win, cap), PAD_NODES, np.int64)  # default: zero row
    slot_dl = np.full((n_win, cap), -1, np.int64)          # -1: empty slot
    for wgl in range(n_win):
        lo, hi = win_bounds[wgl], win_bounds[wgl + 1]
        cnt = hi - lo
        take = min(cnt, cap)
        slot_src[wgl, :take] = src_s[lo : lo + take]
        slot_dl[wgl, :take] = dst_s[lo : lo + take] - wgl * P
        if cnt > cap:
            spill_nodes.append(np.unique(dst_s[lo + cap : hi]))

    indeg = np.bincount(dst, minlength=PAD_NODES)

    # one-hot column indices per slot: col = (tile % TH) * 128 + dst_local
    # shipped layout: [P, WPC * 2 * NIX] int16; per (window, half): NIX entries
    # per partition (tile-within-half 0..TH-1, then one padding -1)
    sl = slot_dl.reshape(n_win, T, P)  # [win, tile, part]
    colix_all = np.full((n_win, 2, NIX, P), -1, np.int64)
    for half in range(2):
        tl = sl[:, half * TH : (half + 1) * TH, :]  # [win, TH, part]
        colix_all[:, half, :TH, :] = np.where(
            tl >= 0, (np.arange(TH)[None, :, None]) * P + tl, -1
        )

    WT = np.ascontiguousarray(W.T).astype(BF16)
    b2 = np.ascontiguousarray(b[:, None])
    iota_np = np.tile(np.arange(P, dtype=np.float32), (P, 1)).astype(BF16)

    in_maps = []
    for c in range(N_CORES):
        wsl = slice(c * WPC, (c + 1) * WPC)
        rows = h_pad_bf[slot_src[wsl]]  # [WPC, cap, HID]
        rows = rows.reshape(WPC, T, P, HID)
        stage_np = np.ascontiguousarray(
            rows.transpose(2, 0, 1, 3).reshape(P, WPC * T * P)
        )
        colix_np = np.ascontiguousarray(
            colix_all[wsl].transpose(3, 0, 1, 2).reshape(P, WPC * 2 * NIX)
        ).astype(np.int16)
        dl_np = np.ascontiguousarray(
            np.where(slot_dl[wsl] >= 0, slot_dl[wsl], 255)
            .reshape(WPC, T, P)
            .transpose(2, 0, 1)
            .reshape(P, WPC * T)
            .astype(np.float32)
        ).astype(BF16)
        hT_np = np.ascontiguousarray(h_pad[c * NPC : (c + 1) * NPC, :].T)
        maskT_np = np.ascontiguousarray(
            np.broadcast_to(
                (indeg[c * NPC : (c + 1) * NPC] > 0).astype(np.uint8)[None, :],
                (P, NPC),
            )
        )
        in_maps.append(
            {
                "stage": stage_np,
                "colix": colix_np,
                "dl": dl_np,
                "iota": iota_np,
                "wt": WT,
                "b2": b2,
                "hT": hT_np,
                "maskT": maskT_np,
            }
        )

    nc = _build_nc()
    res = run_bass_kernel_spmd(nc, in_maps, core_ids=list(range(N_CORES)))

    out = np.concatenate([res.results[c]["outT"].T for c in range(N_CORES)], axis=0)
    out = np.ascontiguousarray(out[:N_NODES])

    # ---- host patch for (statistically negligible) window-capacity spill
    if spill_nodes:
        nodes = np.unique(np.concatenate(spill_nodes))
        nodes = nodes[nodes < N_NODES]
        if nodes.size:
            sel = np.isin(dst, nodes)
            agg = np.zeros((nodes.size, HID), np.float32)
            remap = {int(v): i for i, v in enumerate(nodes)}
            np.add.at(agg, [remap[int(d)] for d in dst[sel]], h[src[sel]])
            out[nodes] = agg @ W.T + b

    return out
